# revision 1
# baseline (speedup 1.0000x reference)
"""Trainium2 distributed kernel for ALRDLinear + 3-bit per-tensor fake-quant.

Reference computation (tokens=8192, in=4096, rank=1024, out=4096, f32):
    y   = input @ B_w.T                       # [tokens, rank]
    y_q = fake_quant(y)                       # per-tensor symmetric 3-bit
    out = y_q @ A_w.T + A_b                   # [tokens, out]

Distribution: data-parallel over tokens across 8 NeuronCores (1024 tok/core).
Weights replicated. The only cross-core dependency is the per-tensor amax,
exchanged with one 64-byte AllGather and reduced locally.

Numerics: matmul1 runs as 3 accumulating fp16 matmul passes on hi/lo splits
(x = xh + xl, B = Bh + Bl; the lo*lo term is dropped) giving ~1e-6 abs error
in y. That precision is required: y feeds round(y/scale), and rounding-boundary
flips are amplified by the 3-bit step size (bf16 or fp32r matmuls fail the
2e-2 gate through this amplification). Quantization uses the +1.5*2^23 RNE
trick with no clip (|round(y/scale)| <= 3 holds by construction of scale);
y_q = q*scale is exact-int times scale stored bf16. Matmul2 runs y_q (bf16)
against bf16 A-weights, bias added on eviction.

Perf notes (measured on TRN2, 8 cores): PE issues N=512 fp16/bf16 matmuls
every ~263ns here; a NEFF containing any collective_compute gets its PE
clock capped ~2.08GHz from load (vs 2.4GHz without — measured, presence-
based), which is why the amax exchange is kept to a single tiny AllGather.
(A remote_dma_broadcast exchange avoids the cap and works functionally,
but its first use per execution waits 4-10ms on lazy init in this
environment.) All cross-core DMA payloads are single-descriptor contiguous
rows: a [128,1] partition-strided bounce costs ~7.5us in 4-byte
descriptors. Phase-2 bulk weight/bias loads are anchored on the amax
bounce DMA so they stream through the otherwise-idle collective-wait
window; matmul2 starts ~4.5us after the AllGather returns.
"""

import numpy as np
import ml_dtypes

P = 128
TOK, IN_F, OUT_F, RANK = 8192, 4096, 4096, 1024
NCORES = 8
TPC = TOK // NCORES            # tokens per core
KT1 = IN_F // P                # 32 contraction tiles for matmul1
MR = RANK // P                 # 8 rank tiles
NT1 = TPC // 512               # 2 token column-tiles in matmul1
MT2 = TPC // P                 # 8 token row-tiles in matmul2
NT2 = OUT_F // 512             # 8 out-feature tiles

QMAX = 3.0
QMIN = -4.0
MAGIC = 1.5 * 2.0**23          # round-to-nearest-even integer trick

_CACHE = {}


def _build():
    import concourse.mybir as mybir
    import concourse.tile as tile
    from concourse import bacc
    from concourse import bass_isa

    nc = bacc.Bacc(None, target_bir_lowering=False, debug=False, num_devices=NCORES)
    f32, f16, bf16 = mybir.dt.float32, mybir.dt.float16, mybir.dt.bfloat16

    xh_d = nc.dram_tensor("xh", [P, NT1, KT1, 512], f16, kind="ExternalInput")
    xl_d = nc.dram_tensor("xl", [P, NT1, KT1, 512], f16, kind="ExternalInput")
    bh_d = nc.dram_tensor("bh", [P, MR, KT1, P], f16, kind="ExternalInput")
    bl_d = nc.dram_tensor("bl", [P, MR, KT1, P], f16, kind="ExternalInput")
    aw_d = nc.dram_tensor("aw", [P, NT2, MR, 512], bf16, kind="ExternalInput")
    bias_d = nc.dram_tensor("bias", [P, OUT_F], f32, kind="ExternalInput")
    out_d = nc.dram_tensor("out", [TPC, OUT_F], f32, kind="ExternalOutput")

    cc_in = nc.dram_tensor("cc_in", [1, 16], f32)
    cc_out = nc.dram_tensor("cc_out", [NCORES, 16], f32, addr_space="Shared")

    ts = lambda i, s: slice(i * s, (i + 1) * s)

    with tile.TileContext(nc) as tc:
        with (
            tc.tile_pool(name="stats", bufs=1) as stats,
            tc.tile_pool(name="ypool", bufs=1) as ypool,
            tc.tile_pool(name="psum", bufs=8, space="PSUM") as psum,
        ):
            y_t = ypool.tile([P, MR, TPC], f32, tag="y")
            am_part = stats.tile([P, MR * NT1], f32, tag="am_part")
            am1 = stats.tile([P, 1], f32, tag="am1")
            am_b = stats.tile([P, 1], f32, tag="am_b")

            # ---------------- phase 1: y.T = B @ x.T (fp16 3-pass) -------
            with (
                tc.tile_pool(name="xpool", bufs=1) as xpool,
                tc.tile_pool(name="bpool", bufs=2) as bpool,
            ):
                xh_t = xpool.tile([P, NT1, KT1, 512], f16, tag="xh")
                xl_t = xpool.tile([P, NT1, KT1, 512], f16, tag="xl")
                # DMAs in consumption order, in chunks with >=4KB contiguous
                # runs per partition (1KB segments starve the stream).
                KG = 4
                bh_t = bpool.tile([P, KT1, P], f16, tag="bh")
                bl_t = bpool.tile([P, KT1, P], f16, tag="bl")
                for g in range(KT1 // KG):
                    sl = ts(g, KG)
                    nc.sync.dma_start(bh_t[:, sl], bh_d[:, 0, sl])
                    nc.sync.dma_start(bl_t[:, sl], bl_d[:, 0, sl])
                    nc.sync.dma_start(xh_t[:, 0, sl], xh_d[:, 0, sl])
                    nc.sync.dma_start(xl_t[:, 0, sl], xl_d[:, 0, sl])
                for g in range(KT1 // KG):
                    nc.sync.dma_start(
                        xh_t[:, 1, ts(g, KG)], xh_d[:, 1, ts(g, KG)])
                    nc.sync.dma_start(
                        xl_t[:, 1, ts(g, KG)], xl_d[:, 1, ts(g, KG)])

                for mr in range(MR):
                    if mr > 0:
                        bh_t = bpool.tile([P, KT1, P], f16, tag="bh")
                        bl_t = bpool.tile([P, KT1, P], f16, tag="bl")
                        nc.sync.dma_start(bh_t[:], bh_d[:, mr])
                        nc.sync.dma_start(bl_t[:], bl_d[:, mr])
                    for nt in range(NT1):
                        ps = psum.tile([P, 512], f32, tag="ps")
                        for k in range(KT1):
                            nc.tensor.matmul(
                                ps[:], bh_t[:, k], xh_t[:, nt, k],
                                start=(k == 0), stop=False)
                            nc.tensor.matmul(
                                ps[:], bh_t[:, k], xl_t[:, nt, k],
                                start=False, stop=False)
                            nc.tensor.matmul(
                                ps[:], bl_t[:, k], xh_t[:, nt, k],
                                start=False, stop=(k == KT1 - 1))
                        idx = mr * NT1 + nt
                        nc.vector.tensor_reduce(
                            am_part[:, idx : idx + 1], ps[:],
                            axis=mybir.AxisListType.X, op=mybir.AluOpType.max,
                            apply_absolute_value=True)
                        nc.scalar.copy(y_t[:, mr, ts(nt, 512)], ps[:])

            # ---------------- amax all-gather + scale ---------------------
            # Keep every cross-core transfer a single contiguous descriptor
            # (a [128,1] partition-strided DMA costs ~7.5us in 4B descriptors).
            nc.vector.tensor_reduce(
                am1[:], am_part[:], axis=mybir.AxisListType.X,
                op=mybir.AluOpType.max)
            nc.gpsimd.partition_all_reduce(
                am_b[:], am1[:], channels=P, reduce_op=bass_isa.ReduceOp.max)
            row16 = stats.tile([1, 16], f32, tag="row16")
            nc.vector.tensor_copy(row16[0:1, :], am_b[0:1, 0:1].to_broadcast([1, 16]))
            bounce_dma = nc.sync.dma_start(cc_in[:, :], row16[0:1, :])
            cc_inst = nc.gpsimd.collective_compute(
                "AllGather", mybir.AluOpType.bypass,
                replica_groups=[list(range(NCORES))],
                ins=[cc_in.ap().opt()], outs=[cc_out.ap().opt()])
            amrow = stats.tile([1, NCORES * 16], f32, tag="amrow")
            ret_dma = nc.sync.dma_start(
                amrow[0:1, :], cc_out[:, :].rearrange("c x -> (c x)")[None, :])
            amg1 = stats.tile([1, 1], f32, tag="amg1")
            nc.vector.tensor_reduce(
                amg1[0:1, :], amrow[0:1, :], axis=mybir.AxisListType.X,
                op=mybir.AluOpType.max)
            # si = [scale, 1/scale] on partition 0; Q7-broadcast to all 128
            si = stats.tile([1, 2], f32, tag="si")
            nc.vector.tensor_scalar(
                si[0:1, 0:1], amg1[0:1, :], 1e-8, float(np.float32(1.0 / QMAX)),
                mybir.AluOpType.max, mybir.AluOpType.mult)
            nc.vector.reciprocal(si[0:1, 1:2], si[0:1, 0:1])
            bc = stats.tile([P, 2], f32, tag="bc")
            nc.gpsimd.partition_broadcast(bc[:], si[0:1, :], channels=P)
            scale_t = bc[:, 0:1]
            inv_t = bc[:, 1:2]

            # ---------------- phase 2: quant + out = q @ (Aw*scale) + b --
            with (
                tc.tile_pool(name="qpool", bufs=1) as qpool,
                tc.tile_pool(name="tpool", bufs=1) as tpool,
                tc.tile_pool(name="apool", bufs=2) as apool,
                tc.tile_pool(name="opool", bufs=4) as opool,
                tc.tile_pool(name="biasp", bufs=1) as biasp,
            ):
                from concourse.tile_rust import add_dep_helper

                # Phase-2 bulk loads would otherwise be released exactly at
                # mm1-end (their SBUF overlaps the freed x pool) and their
                # queue drain delays the tiny amax bounce DMA by ~10us. Gate
                # them behind the collective's return DMA instead.
                bias_t = biasp.tile([P, OUT_F], f32, tag="bias")
                bias_dma = nc.sync.dma_start(bias_t[:], bias_d[:, :])
                add_dep_helper(bias_dma.ins, bounce_dma.ins,
                               reason="stream during collective wait")

                # No explicit clip needed: scale = amax/QMAX with amax taken
                # over these same y values, so |round(y*inv)| <= QMAX always
                # (and QMIN=-4 < -QMAX is unreachable for symmetric data).
                q_t = qpool.tile([P, MR, TPC], bf16, tag="q")
                for mt in range(MT2):
                    sl = ts(mt, P)
                    krs = [(0, 4), (4, 8)] if mt == 0 else [(0, MR)]
                    for k0, k1 in krs:
                        t1 = tpool.tile([P, MR, P], f32, tag="t1")
                        # t1 = y*inv + MAGIC  (RNE in the f32 lattice)
                        nc.vector.tensor_scalar(
                            t1[:, k0:k1], y_t[:, k0:k1, sl], inv_t[:], MAGIC,
                            mybir.AluOpType.mult, mybir.AluOpType.add)
                        # y_q = (t1 - MAGIC) * scale -> bf16 (8 exact levels)
                        nc.vector.tensor_scalar(
                            q_t[:, k0:k1, sl], t1[:, k0:k1], -MAGIC, scale_t[:],
                            mybir.AluOpType.add, mybir.AluOpType.mult)

                last_a0 = None
                for nt in range(NT2):
                    a_t = apool.tile([P, MR, 512], bf16, tag="aw")
                    if nt <= 1:
                        a_dma = nc.sync.dma_start(a_t[:], aw_d[:, nt])
                        add_dep_helper(a_dma.ins, bounce_dma.ins,
                                       reason="stream during collective wait")
                    else:
                        a_dma = nc.sync.dma_start(a_t[:], aw_d[:, nt])
                    for mt in range(MT2):
                        ps2 = psum.tile([P, 512], f32, tag="ps")
                        for kr in range(MR):
                            nc.tensor.matmul(
                                ps2[:], q_t[:, kr, ts(mt, P)], a_t[:, kr],
                                start=(kr == 0), stop=(kr == MR - 1))
                        o_t = opool.tile([P, 512], f32, tag="o")
                        nc.vector.tensor_tensor(
                            o_t[:], ps2[:], bias_t[:, ts(nt, 512)],
                            mybir.AluOpType.add)
                        nc.sync.dma_start(out_d[ts(mt, P), ts(nt, 512)], o_t[:])

    nc.compile()
    return nc


def _get_nc():
    if "nc" not in _CACHE:
        _CACHE["nc"] = _build()
    return _CACHE["nc"]


def kernel(input, B_w, A_w, A_b):
    from concourse import bass_utils

    nc = _get_nc()

    f32 = np.float32
    bf16 = ml_dtypes.bfloat16

    # Accept jax arrays / non-contiguous inputs
    input = np.asarray(input, dtype=f32)
    B_w = np.asarray(B_w, dtype=f32)
    A_w = np.asarray(A_w, dtype=f32)
    A_b = np.asarray(A_b, dtype=f32)

    # Weights (replicated, pre-laid-out for the PE's [K-on-partitions] form).
    BwT = np.ascontiguousarray(B_w.astype(f32, copy=False).T)     # [IN_F, RANK]
    Bh = BwT.astype(np.float16)
    Bl = (BwT - Bh.astype(f32)).astype(np.float16)
    Bh = np.ascontiguousarray(Bh.reshape(KT1, P, MR, P).transpose(1, 2, 0, 3))
    Bl = np.ascontiguousarray(Bl.reshape(KT1, P, MR, P).transpose(1, 2, 0, 3))

    AwT = np.ascontiguousarray(A_w.astype(f32, copy=False).T)     # [RANK, OUT_F]
    Aw = np.ascontiguousarray(
        AwT.astype(bf16).reshape(MR, P, NT2, 512).transpose(1, 2, 0, 3))

    bias_rep = np.ascontiguousarray(
        np.broadcast_to(A_b.astype(f32, copy=False), (P, OUT_F)))

    in_maps = []
    for c in range(NCORES):
        xT = np.ascontiguousarray(input[c * TPC : (c + 1) * TPC].astype(f32, copy=False).T)
        xh = xT.astype(np.float16)
        xl = (xT - xh.astype(f32)).astype(np.float16)
        # [IN_F, TPC] -> [P, NT1, KT1, 512]: per-partition contiguous chunks
        xh = np.ascontiguousarray(
            xh.reshape(KT1, P, NT1, 512).transpose(1, 2, 0, 3))
        xl = np.ascontiguousarray(
            xl.reshape(KT1, P, NT1, 512).transpose(1, 2, 0, 3))
        in_maps.append(
            {"xh": xh, "xl": xl, "bh": Bh, "bl": Bl, "aw": Aw, "bias": bias_rep}
        )

    res = bass_utils.run_bass_kernel_spmd(nc, in_maps, core_ids=list(range(NCORES)))
    out = np.concatenate([res.results[c]["out"] for c in range(NCORES)], axis=0)
    return out.astype(np.float32, copy=False)



# revision 5
# speedup vs baseline: 1.0053x; 1.0053x over previous
"""Trainium2 distributed kernel for ALRDLinear + 3-bit per-tensor fake-quant.

Reference computation (tokens=8192, in=4096, rank=1024, out=4096, f32):
    y   = input @ B_w.T                       # [tokens, rank]
    y_q = fake_quant(y)                       # per-tensor symmetric 3-bit
    out = y_q @ A_w.T + A_b                   # [tokens, out]

Distribution: data-parallel over tokens across 8 NeuronCores (1024 tok/core).
Weights replicated. The only cross-core dependency is the per-tensor amax,
exchanged with one 64-byte AllGather and reduced locally.

Numerics: y needs ~fp16x2 precision because it feeds round(y/scale) and
rounding-boundary flips are amplified by the 3-bit step (bf16 or fp32r
matmuls fail the 2e-2 gate). matmul1 = one fp16 main pass (Bh@xh) plus ONE
fp8-DoubleRow pass computing BOTH correction terms fused: each PE cell holds
the weight pair [Bh8, Bl8] and streams the pair [xl8, x8], accumulating
Bh@xl + Bl@x at 2x contraction rate (K=256/instruction). All operands are
pre-scaled so every product lands in the same PSUM at 2^15 scale; the 2^-15
is folded into the scalar scale/inv computation after the amax AllGather.
Measured y error std ~1e-5 -> final rel err ~3.5e-3 (gate 2e-2).

Quantization uses the +1.5*2^23 RNE trick with no clip (|q| <= 3 by
construction of scale); y_q = q*scale stored bf16. Matmul2 runs y_q (bf16)
against bf16 A-weights, bias added on eviction.

Perf notes (measured on TRN2): PE issues N=512 matmuls every ~263ns at the
collective-capped 2.08GHz clock; fp8 DoubleRow issues at the same per-MM
rate (2x work). Loop order is k-outer/mr-inner for the fp16 sweep and
mr-outer/k-inner for the DR sweep so each streamed x-chunk is consumed by
all 8 rank tiles immediately (~180GB/s just-in-time demand instead of the
~475GB/s a mr-outer fp16 sweep would need, which starved the PE for ~30us).
B-weights are re-streamed per token-tile (extra 16MB, free bandwidth-wise)
so SBUF holds both token-tiles of x. A dummy AllGather issued at kernel
start warms the collective path/absorbs core launch skew off the critical
path; the real 64B amax AllGather then completes in ~10-20us instead of
~29us. Phase-2 bulk weight/bias loads are gated on the amax bounce DMA so
they stream through the collective-wait window.
"""

import numpy as np
import ml_dtypes

P = 128
TOK, IN_F, OUT_F, RANK = 8192, 4096, 4096, 1024
NCORES = 8
TPC = TOK // NCORES            # tokens per core
KT1 = IN_F // P                # 32 contraction tiles for matmul1
MR = RANK // P                 # 8 rank tiles
NT1 = TPC // 512               # 2 token column-tiles in matmul1
MT2 = TPC // P                 # 8 token row-tiles in matmul2
NT2 = OUT_F // 512             # 8 out-feature tiles

QMAX = 3.0
MAGIC = 1.5 * 2.0**23          # round-to-nearest-even integer trick
EPS_PS = 1e-8 * 2.0**15        # reference's 1e-8 floor, at psum scale

_CACHE = {}


def _build():
    import concourse.mybir as mybir
    import concourse.tile as tile
    from concourse import bacc
    from concourse import bass_isa

    nc = bacc.Bacc(None, target_bir_lowering=False, debug=False, num_devices=NCORES)
    f32, f16, bf16 = mybir.dt.float32, mybir.dt.float16, mybir.dt.bfloat16
    f8 = mybir.dt.float8e4
    DR = mybir.MatmulPerfMode.DoubleRow

    xh_d = nc.dram_tensor("xh", [P, NT1, KT1, 512], f16, kind="ExternalInput")
    m8_d = nc.dram_tensor("m8", [P, NT1, KT1, 2, 512], f8, kind="ExternalInput")
    bh_d = nc.dram_tensor("bh", [P, KT1, MR, P], f16, kind="ExternalInput")
    w8_d = nc.dram_tensor("w8", [P, KT1, MR, 2, P], f8, kind="ExternalInput")
    aw_d = nc.dram_tensor("aw", [P, NT2, MR, 512], bf16, kind="ExternalInput")
    bias_d = nc.dram_tensor("bias", [P, OUT_F], f32, kind="ExternalInput")
    out_d = nc.dram_tensor("out", [TPC, OUT_F], f32, kind="ExternalOutput")

    cc_in = nc.dram_tensor("cc_in", [1, 16], f32)
    cc_out = nc.dram_tensor("cc_out", [NCORES, 16], f32, addr_space="Shared")
    cc_ind = nc.dram_tensor("cc_ind", [1, 16], f32)
    cc_outd = nc.dram_tensor("cc_outd", [NCORES, 16], f32, addr_space="Shared")

    ts = lambda i, s: slice(i * s, (i + 1) * s)

    from concourse.tile_rust import add_dep_helper

    with tile.TileContext(nc) as tc:
        with (
            tc.tile_pool(name="stats", bufs=1) as stats,
            tc.tile_pool(name="ypool", bufs=1) as ypool,
            tc.tile_pool(name="psum", bufs=8, space="PSUM") as psum,
        ):
            y_t = ypool.tile([P, MR, TPC], f32, tag="y")
            am_part = stats.tile([P, MR * NT1], f32, tag="am_part")
            am1 = stats.tile([P, 1], f32, tag="am1")
            am_b = stats.tile([P, 1], f32, tag="am_b")

            # Dummy AllGather: warms the collective path (ring setup, core
            # launch-skew absorption) while the PE crunches matmul1. No
            # return DMA -- nothing consumes it; the real collective queues
            # behind it on the CC stream.
            drow = stats.tile([1, 16], f32, tag="drow")
            nc.vector.memset(drow[0:1, :], 1.0)
            nc.sync.dma_start(cc_ind[:, :], drow[0:1, :])
            cc_dummy = nc.gpsimd.collective_compute(
                "AllGather", mybir.AluOpType.bypass,
                replica_groups=[list(range(NCORES))],
                ins=[cc_ind.ap().opt()], outs=[cc_outd.ap().opt()])

            # ---------------- phase 1: y.T = B @ x.T ---------------------
            # per nt: sweep1 = fp16 main, sweep2 = fp8 DoubleRow corrections.
            # Both sweeps run k-outer/mr-inner so each just-in-time x chunk
            # is consumed by all 8 rank tiles immediately (~180GB/s demand);
            # sweep2's last KTAIL k-tiles run mr-outer so the 8 psum tiles
            # close staggered and their evictions hide under PE work.
            KG = 4       # x-chunk granularity (4KB per partition)
            KTAIL = 8    # k-tiles in the staggered-eviction tail
            KB = KT1 - KTAIL
            with (
                tc.tile_pool(name="xpool", bufs=1) as xpool,
                tc.tile_pool(name="bpool", bufs=2) as bpool,
                tc.tile_pool(name="wpool", bufs=2) as wpool,
                tc.tile_pool(name="wtpool", bufs=1) as wtpool,
            ):
                xh_t = xpool.tile([P, NT1, KT1, 512], f16, tag="xh")
                m8_t = xpool.tile([P, NT1, KT1, 2, 512], f8, tag="m8")
                w8tail_t = wtpool.tile([P, KTAIL, MR, 2, P], f8, tag="w8tail")

                for nt in range(NT1):
                    ps = []
                    for mr in range(MR):
                        pst = psum.tile([P, 512], f32, tag="ps",
                                        name=f"ps_{nt}_{mr}")
                        ps.append(pst)
                    # sweep1: fp16 main pass (k-outer); DMA triggers are
                    # emitted inline in consumption order
                    for k in range(KT1):
                        if k % KG == 0:
                            g = k // KG
                            nc.sync.dma_start(
                                xh_t[:, nt, ts(g, KG)], xh_d[:, nt, ts(g, KG)])
                        bh_t = bpool.tile([P, MR, P], f16, tag="bh")
                        nc.sync.dma_start(bh_t[:], bh_d[:, k])
                        for mr in range(MR):
                            nc.tensor.matmul(
                                ps[mr][:], bh_t[:, mr], xh_t[:, nt, k],
                                start=(k == 0), stop=False)
                    # sweep2 head: DR corrections, k-outer
                    for k in range(KB):
                        if k % KG == 0:
                            g = k // KG
                            nc.sync.dma_start(
                                m8_t[:, nt, ts(g, KG)], m8_d[:, nt, ts(g, KG)])
                        if nt == 0 and k < 2 * KTAIL and k % 2 == 1:
                            j = k // 2
                            nc.sync.dma_start(
                                w8tail_t[:, j : j + 1], w8_d[:, KB + j : KB + j + 1])
                        w8_t = wpool.tile([P, MR, 2, P], f8, tag="w8")
                        nc.sync.dma_start(w8_t[:], w8_d[:, k])
                        for mr in range(MR):
                            nc.tensor.matmul(
                                ps[mr][:], w8_t[:, mr], m8_t[:, nt, k],
                                start=False, stop=False, perf_mode=DR)
                    # sweep2 tail: mr-outer, staggered psum close + eviction
                    for g in range(KB // KG, KT1 // KG):
                        nc.sync.dma_start(
                            m8_t[:, nt, ts(g, KG)], m8_d[:, nt, ts(g, KG)])
                    for mr in range(MR):
                        for k in range(KB, KT1):
                            nc.tensor.matmul(
                                ps[mr][:], w8tail_t[:, k - KB, mr],
                                m8_t[:, nt, k],
                                start=False, stop=(k == KT1 - 1),
                                perf_mode=DR)
                        idx = nt * MR + mr
                        nc.vector.tensor_reduce(
                            am_part[:, idx : idx + 1], ps[mr][:],
                            axis=mybir.AxisListType.X, op=mybir.AluOpType.max,
                            apply_absolute_value=True)
                        nc.scalar.copy(y_t[:, mr, ts(nt, 512)], ps[mr][:])

            # ---------------- amax all-gather + scale ---------------------
            # Keep every cross-core transfer a single contiguous descriptor
            # (a [128,1] partition-strided DMA costs ~7.5us in 4B descriptors).
            nc.vector.tensor_reduce(
                am1[:], am_part[:], axis=mybir.AxisListType.X,
                op=mybir.AluOpType.max)
            nc.gpsimd.partition_all_reduce(
                am_b[:], am1[:], channels=P, reduce_op=bass_isa.ReduceOp.max)
            row16 = stats.tile([1, 16], f32, tag="row16")
            nc.vector.tensor_copy(row16[0:1, :], am_b[0:1, 0:1].to_broadcast([1, 16]))
            bounce_dma = nc.sync.dma_start(cc_in[:, :], row16[0:1, :])
            cc_inst = nc.gpsimd.collective_compute(
                "AllGather", mybir.AluOpType.bypass,
                replica_groups=[list(range(NCORES))],
                ins=[cc_in.ap().opt()], outs=[cc_out.ap().opt()])
            add_dep_helper(cc_inst.ins, cc_dummy.ins,
                           reason="real collective after warmup dummy")
            amrow = stats.tile([1, NCORES * 16], f32, tag="amrow")
            ret_dma = nc.sync.dma_start(
                amrow[0:1, :], cc_out[:, :].rearrange("c x -> (c x)")[None, :])
            amg1 = stats.tile([1, 1], f32, tag="amg1")
            nc.vector.tensor_reduce(
                amg1[0:1, :], amrow[0:1, :], axis=mybir.AxisListType.X,
                op=mybir.AluOpType.max)
            # si = [u, 1/u, u*2^-15] with u = max(amax_ps, eps)/QMAX:
            # 1/u is the quant multiplier for psum-scaled y; u*2^-15 is the
            # true dequant scale.
            si = stats.tile([1, 3], f32, tag="si")
            nc.vector.tensor_scalar(
                si[0:1, 0:1], amg1[0:1, :], EPS_PS, float(np.float32(1.0 / QMAX)),
                mybir.AluOpType.max, mybir.AluOpType.mult)
            nc.vector.reciprocal(si[0:1, 1:2], si[0:1, 0:1])
            nc.vector.tensor_scalar(
                si[0:1, 2:3], si[0:1, 0:1], float(np.float32(2.0**-15)), None,
                mybir.AluOpType.mult)
            bc = stats.tile([P, 3], f32, tag="bc")
            nc.gpsimd.partition_broadcast(bc[:], si[0:1, :], channels=P)
            inv_t = bc[:, 1:2]
            scale_t = bc[:, 2:3]

            # ---------------- phase 2: quant + out = q @ Aw + b -----------
            with (
                tc.tile_pool(name="qpool", bufs=1) as qpool,
                tc.tile_pool(name="tpool", bufs=1) as tpool,
                tc.tile_pool(name="apool", bufs=2) as apool,
                tc.tile_pool(name="opool", bufs=4) as opool,
                tc.tile_pool(name="biasp", bufs=1) as biasp,
            ):
                # Phase-2 bulk loads would otherwise be released exactly at
                # mm1-end and their queue drain delays the tiny amax bounce
                # DMA. Gate them behind the bounce so they stream during the
                # collective wait instead.
                bias_t = biasp.tile([P, OUT_F], f32, tag="bias")
                bias_dma = nc.sync.dma_start(bias_t[:], bias_d[:, :])
                add_dep_helper(bias_dma.ins, bounce_dma.ins,
                               reason="stream during collective wait")

                # No explicit clip needed: scale = amax/QMAX with amax taken
                # over these same y values, so |round(y*inv)| <= QMAX always.
                q_t = qpool.tile([P, MR, TPC], bf16, tag="q")
                for mt in range(MT2):
                    sl = ts(mt, P)
                    krs = [(0, 4), (4, 8)] if mt == 0 else [(0, MR)]
                    for k0, k1 in krs:
                        t1 = tpool.tile([P, MR, P], f32, tag="t1")
                        # t1 = y*inv + MAGIC  (RNE in the f32 lattice)
                        nc.vector.tensor_scalar(
                            t1[:, k0:k1], y_t[:, k0:k1, sl], inv_t[:], MAGIC,
                            mybir.AluOpType.mult, mybir.AluOpType.add)
                        # y_q = (t1 - MAGIC) * scale -> bf16 (8 exact levels)
                        nc.vector.tensor_scalar(
                            q_t[:, k0:k1, sl], t1[:, k0:k1], -MAGIC, scale_t[:],
                            mybir.AluOpType.add, mybir.AluOpType.mult)

                for nt in range(NT2):
                    a_t = apool.tile([P, MR, 512], bf16, tag="aw")
                    a_dma = nc.sync.dma_start(a_t[:], aw_d[:, nt])
                    if nt <= 1:
                        add_dep_helper(a_dma.ins, bounce_dma.ins,
                                       reason="stream during collective wait")
                    for mt in range(MT2):
                        ps2 = psum.tile([P, 512], f32, tag="ps")
                        for kr in range(MR):
                            nc.tensor.matmul(
                                ps2[:], q_t[:, kr, ts(mt, P)], a_t[:, kr],
                                start=(kr == 0), stop=(kr == MR - 1))
                        o_t = opool.tile([P, 512], f32, tag="o")
                        nc.vector.tensor_tensor(
                            o_t[:], ps2[:], bias_t[:, ts(nt, 512)],
                            mybir.AluOpType.add)
                        nc.scalar.dma_start(out_d[ts(mt, P), ts(nt, 512)], o_t[:])

    nc.compile()
    return nc


def _get_nc():
    if "nc" not in _CACHE:
        _CACHE["nc"] = _build()
    return _CACHE["nc"]


def kernel(input, B_w, A_w, A_b):
    from concourse import bass_utils

    nc = _get_nc()

    f32 = np.float32
    f16 = np.float16
    bf16 = ml_dtypes.bfloat16
    e4 = ml_dtypes.float8_e4m3

    # Accept jax arrays / non-contiguous inputs
    input = np.asarray(input, dtype=f32)
    B_w = np.asarray(B_w, dtype=f32)
    A_w = np.asarray(A_w, dtype=f32)
    A_b = np.asarray(A_b, dtype=f32)

    # Weights, pre-scaled for the shared-psum 2^15 convention:
    #   main:  (Bh*2^8 fp16) @ (xh*2^7 fp16)
    #   corr:  (Bh*2^6 fp8) @ (xl*2^9 fp8)  +  (Bl*2^17 fp8) @ (x*2^-2 fp8)
    BT = np.ascontiguousarray(B_w.T).astype(f32) * np.float32(2.0**8)
    Bh16 = BT.astype(f16)                       # [IN_F, RANK] at 2^8
    Bl_s = BT - Bh16.astype(f32)
    w0 = (Bh16.astype(f32) * np.float32(0.25)).astype(e4)
    w1 = (Bl_s * np.float32(512.0)).astype(e4)
    bh = np.ascontiguousarray(
        Bh16.reshape(KT1, P, MR, P).transpose(1, 0, 2, 3))
    w0p = w0.reshape(KT1, P, MR, P).transpose(1, 0, 2, 3)
    w1p = w1.reshape(KT1, P, MR, P).transpose(1, 0, 2, 3)
    w8 = np.ascontiguousarray(np.stack([w0p, w1p], axis=3))  # [P,KT1,MR,2,P]

    AwT = np.ascontiguousarray(A_w.T)                        # [RANK, OUT_F]
    Aw = np.ascontiguousarray(
        AwT.astype(bf16).reshape(MR, P, NT2, 512).transpose(1, 2, 0, 3))

    bias_rep = np.ascontiguousarray(
        np.broadcast_to(A_b.astype(f32, copy=False), (P, OUT_F)))

    in_maps = []
    for c in range(NCORES):
        xT = np.ascontiguousarray(input[c * TPC : (c + 1) * TPC].T).astype(f32)
        xTs = xT * np.float32(2.0**7)
        xh16 = xTs.astype(f16)
        xl_s = xTs - xh16.astype(f32)
        m0 = (xl_s * np.float32(4.0)).astype(e4)
        m1 = (xTs * np.float32(2.0**-9)).astype(e4)
        xh = np.ascontiguousarray(
            xh16.reshape(KT1, P, NT1, 512).transpose(1, 2, 0, 3))
        m0p = m0.reshape(KT1, P, NT1, 512).transpose(1, 2, 0, 3)
        m1p = m1.reshape(KT1, P, NT1, 512).transpose(1, 2, 0, 3)
        m8 = np.ascontiguousarray(np.stack([m0p, m1p], axis=3))
        in_maps.append(
            {"xh": xh, "m8": m8, "bh": bh, "w8": w8, "aw": Aw, "bias": bias_rep}
        )

    res = bass_utils.run_bass_kernel_spmd(nc, in_maps, core_ids=list(range(NCORES)))
    out = np.concatenate([res.results[c]["out"] for c in range(NCORES)], axis=0)
    return out.astype(np.float32, copy=False)


# revision 6
# speedup vs baseline: 1.3069x; 1.3000x over previous
"""Trainium2 distributed kernel for ALRDLinear + 3-bit per-tensor fake-quant.

Reference computation (tokens=8192, in=4096, rank=1024, out=4096, f32):
    y   = input @ B_w.T                       # [tokens, rank]
    y_q = fake_quant(y)                       # per-tensor symmetric 3-bit
    out = y_q @ A_w.T + A_b                   # [tokens, out]

Distribution: data-parallel over tokens across 8 NeuronCores (1024 tok/core).
Weights replicated. The only cross-core dependency is the per-tensor amax,
exchanged with one 64-byte AllGather and reduced locally.

Numerics: y needs ~fp16x2 precision because it feeds round(y/scale) and
rounding-boundary flips are amplified by the 3-bit step (bf16 or fp32r
matmuls fail the 2e-2 gate). matmul1 = one fp16 main pass (Bh@xh) plus ONE
fp8-DoubleRow pass computing BOTH correction terms fused: each PE cell holds
the weight pair [Bh8, Bl8] and streams the pair [xl8, x8], accumulating
Bh@xl + Bl@x at 2x contraction rate (K=256/instruction). All operands are
pre-scaled so every product lands in the same PSUM at 2^15 scale; the 2^-15
is folded into the scalar scale/inv computation after the amax AllGather.
Measured y error std ~1e-5 -> final rel err ~3.5e-3 (gate 2e-2).

Quantization uses the +1.5*2^23 RNE trick with no clip (|q| <= 3 by
construction of scale); y_q = q*scale stored bf16. Matmul2 runs y_q (bf16)
against bf16 A-weights, bias added on eviction.

Perf notes (measured on TRN2): PE issues N=512 matmuls every ~263ns at the
collective-capped 2.08GHz clock; fp8 DoubleRow issues at the same per-MM
rate (2x work). Loop order is k-outer/mr-inner for the fp16 sweep and
mr-outer/k-inner for the DR sweep so each streamed x-chunk is consumed by
all 8 rank tiles immediately (~180GB/s just-in-time demand instead of the
~475GB/s a mr-outer fp16 sweep would need, which starved the PE for ~30us).
B-weights are re-streamed per token-tile (extra 16MB, free bandwidth-wise)
so SBUF holds both token-tiles of x. A dummy AllGather issued at kernel
start warms the collective path/absorbs core launch skew off the critical
path; the real 64B amax AllGather then completes in ~10-20us instead of
~29us. Phase-2 bulk weight/bias loads are gated on the amax bounce DMA so
they stream through the collective-wait window.
"""

import numpy as np
import ml_dtypes

P = 128
TOK, IN_F, OUT_F, RANK = 8192, 4096, 4096, 1024
NCORES = 8
TPC = TOK // NCORES            # tokens per core
KT1 = IN_F // P                # 32 contraction tiles for matmul1
MR = RANK // P                 # 8 rank tiles
NT1 = TPC // 512               # 2 token column-tiles in matmul1
MT2 = TPC // P                 # 8 token row-tiles in matmul2
NT2 = OUT_F // 512             # 8 out-feature tiles

QMAX = 3.0
MAGIC = 1.5 * 2.0**23          # round-to-nearest-even integer trick
EPS_PS = 1e-8 * 2.0**15        # reference's 1e-8 floor, at psum scale

_CACHE = {}


def _build():
    import concourse.mybir as mybir
    import concourse.tile as tile
    from concourse import bacc
    from concourse import bass_isa

    nc = bacc.Bacc(None, target_bir_lowering=False, debug=False, num_devices=NCORES)
    f32, f16, bf16 = mybir.dt.float32, mybir.dt.float16, mybir.dt.bfloat16
    f8 = mybir.dt.float8e4
    DR = mybir.MatmulPerfMode.DoubleRow

    xh_d = nc.dram_tensor("xh", [P, NT1, KT1, 512], f16, kind="ExternalInput")
    m8_d = nc.dram_tensor("m8", [P, NT1, KT1, 2, 512], f8, kind="ExternalInput")
    bh_d = nc.dram_tensor("bh", [P, KT1, MR, P], f16, kind="ExternalInput")
    w8_d = nc.dram_tensor("w8", [P, KT1, MR, 2, P], f8, kind="ExternalInput")
    aw_d = nc.dram_tensor("aw", [P, NT2, MR, 512], bf16, kind="ExternalInput")
    bias_d = nc.dram_tensor("bias", [P, OUT_F], f32, kind="ExternalInput")
    out_d = nc.dram_tensor("out", [TPC, OUT_F], f32, kind="ExternalOutput")

    cc_in = nc.dram_tensor("cc_in", [1, 16], f32)
    cc_out = nc.dram_tensor("cc_out", [NCORES, 16], f32, addr_space="Shared")
    cc_ind = nc.dram_tensor("cc_ind", [1, 16], f32)
    cc_outd = nc.dram_tensor("cc_outd", [NCORES, 16], f32, addr_space="Shared")

    ts = lambda i, s: slice(i * s, (i + 1) * s)

    from concourse.tile_rust import add_dep_helper

    with tile.TileContext(nc) as tc:
        with (
            tc.tile_pool(name="stats", bufs=1) as stats,
            tc.tile_pool(name="ypool", bufs=1) as ypool,
            tc.tile_pool(name="psum", bufs=8, space="PSUM") as psum,
        ):
            y_t = ypool.tile([P, MR, TPC], f32, tag="y")
            am_part = stats.tile([P, MR * NT1], f32, tag="am_part")
            am1 = stats.tile([P, 1], f32, tag="am1")
            am_b = stats.tile([P, 1], f32, tag="am_b")

            # Dummy AllGather: warms the collective path (ring setup, core
            # launch-skew absorption) while the PE crunches matmul1. No
            # return DMA -- nothing consumes it; the real collective queues
            # behind it on the CC stream.
            drow = stats.tile([1, 16], f32, tag="drow")
            nc.vector.memset(drow[0:1, :], 1.0)
            nc.sync.dma_start(cc_ind[:, :], drow[0:1, :])
            cc_dummy = nc.gpsimd.collective_compute(
                "AllGather", mybir.AluOpType.bypass,
                replica_groups=[list(range(NCORES))],
                ins=[cc_ind.ap().opt()], outs=[cc_outd.ap().opt()])

            # ---------------- phase 1: y.T = B @ x.T ---------------------
            # per nt: sweep1 = fp16 main, sweep2 = fp8 DoubleRow corrections.
            # Both sweeps run k-outer/mr-inner so each just-in-time x chunk
            # is consumed by all 8 rank tiles immediately (~180GB/s demand);
            # sweep2's last KTAIL k-tiles run mr-outer so the 8 psum tiles
            # close staggered and their evictions hide under PE work.
            KG = 4       # k-tiles per streamed weight/x chunk
            NG = KT1 // KG
            KTAIL = 4    # k-tiles in the staggered-eviction tail
            KB = KT1 - KTAIL
            with (
                tc.tile_pool(name="xpool", bufs=1) as xpool,
                tc.tile_pool(name="bpool", bufs=2) as bpool,
                tc.tile_pool(name="wpool", bufs=2) as wpool,
                tc.tile_pool(name="wtpool", bufs=1) as wtpool,
            ):
                xh_t = xpool.tile([P, NT1, KT1, 512], f16, tag="xh")
                m8_t = xpool.tile([P, NT1, KT1, 2, 512], f8, tag="m8")
                w8tail_t = wtpool.tile([P, KTAIL, MR, 2, P], f8, tag="w8tail")

                for nt in range(NT1):
                    ps = []
                    for mr in range(MR):
                        pst = psum.tile([P, 512], f32, tag="ps",
                                        name=f"ps_{nt}_{mr}")
                        ps.append(pst)
                    # sweep1: fp16 main pass (k-outer); weight groups are
                    # 1MB double-buffered transfers, x chunks lead by 2
                    nc.sync.dma_start(xh_t[:, nt, ts(0, KG)], xh_d[:, nt, ts(0, KG)])
                    nc.sync.dma_start(xh_t[:, nt, ts(1, KG)], xh_d[:, nt, ts(1, KG)])
                    for g in range(NG):
                        if g + 2 < NG:
                            nc.sync.dma_start(
                                xh_t[:, nt, ts(g + 2, KG)],
                                xh_d[:, nt, ts(g + 2, KG)])
                        bh_t = bpool.tile([P, KG, MR, P], f16, tag="bh")
                        nc.sync.dma_start(bh_t[:], bh_d[:, ts(g, KG)])
                        if nt == 0 and g in (4, 5):
                            j = g - 4
                            h = KTAIL // 2
                            nc.sync.dma_start(
                                w8tail_t[:, ts(j, h)], w8_d[:, KB + j * h : KB + (j + 1) * h])
                        for kk in range(KG):
                            k = g * KG + kk
                            for mr in range(MR):
                                nc.tensor.matmul(
                                    ps[mr][:], bh_t[:, kk, mr], xh_t[:, nt, k],
                                    start=(k == 0), stop=False)
                    # sweep2 head: DR corrections, k-outer over k < KB
                    nc.sync.dma_start(m8_t[:, nt, ts(0, KG)], m8_d[:, nt, ts(0, KG)])
                    nc.sync.dma_start(m8_t[:, nt, ts(1, KG)], m8_d[:, nt, ts(1, KG)])
                    for g in range(KB // KG):
                        if g + 2 < NG:
                            nc.sync.dma_start(
                                m8_t[:, nt, ts(g + 2, KG)],
                                m8_d[:, nt, ts(g + 2, KG)])
                        w8_t = wpool.tile([P, KG, MR, 2, P], f8, tag="w8")
                        nc.sync.dma_start(w8_t[:], w8_d[:, ts(g, KG)])
                        for kk in range(KG):
                            k = g * KG + kk
                            for mr in range(MR):
                                nc.tensor.matmul(
                                    ps[mr][:], w8_t[:, kk, mr], m8_t[:, nt, k],
                                    start=False, stop=False, perf_mode=DR)
                    # sweep2 tail: mr-outer, staggered psum close + eviction
                    for mr in range(MR):
                        for k in range(KB, KT1):
                            nc.tensor.matmul(
                                ps[mr][:], w8tail_t[:, k - KB, mr],
                                m8_t[:, nt, k],
                                start=False, stop=(k == KT1 - 1),
                                perf_mode=DR)
                        idx = nt * MR + mr
                        nc.vector.tensor_reduce(
                            am_part[:, idx : idx + 1], ps[mr][:],
                            axis=mybir.AxisListType.X, op=mybir.AluOpType.max,
                            apply_absolute_value=True)
                        nc.scalar.copy(y_t[:, mr, ts(nt, 512)], ps[mr][:])

            # ---------------- amax all-gather + scale ---------------------
            # Keep every cross-core transfer a single contiguous descriptor
            # (a [128,1] partition-strided DMA costs ~7.5us in 4B descriptors).
            nc.vector.tensor_reduce(
                am1[:], am_part[:], axis=mybir.AxisListType.X,
                op=mybir.AluOpType.max)
            nc.gpsimd.partition_all_reduce(
                am_b[:], am1[:], channels=P, reduce_op=bass_isa.ReduceOp.max)
            row16 = stats.tile([1, 16], f32, tag="row16")
            nc.vector.tensor_copy(row16[0:1, :], am_b[0:1, 0:1].to_broadcast([1, 16]))
            bounce_dma = nc.sync.dma_start(cc_in[:, :], row16[0:1, :])
            cc_inst = nc.gpsimd.collective_compute(
                "AllGather", mybir.AluOpType.bypass,
                replica_groups=[list(range(NCORES))],
                ins=[cc_in.ap().opt()], outs=[cc_out.ap().opt()])
            add_dep_helper(cc_inst.ins, cc_dummy.ins,
                           reason="real collective after warmup dummy")
            amrow = stats.tile([1, NCORES * 16], f32, tag="amrow")
            ret_dma = nc.sync.dma_start(
                amrow[0:1, :], cc_out[:, :].rearrange("c x -> (c x)")[None, :])
            amg1 = stats.tile([1, 1], f32, tag="amg1")
            nc.vector.tensor_reduce(
                amg1[0:1, :], amrow[0:1, :], axis=mybir.AxisListType.X,
                op=mybir.AluOpType.max)
            # si = [u, 1/u, u*2^-15] with u = max(amax_ps, eps)/QMAX:
            # 1/u is the quant multiplier for psum-scaled y; u*2^-15 is the
            # true dequant scale.
            si = stats.tile([1, 3], f32, tag="si")
            nc.vector.tensor_scalar(
                si[0:1, 0:1], amg1[0:1, :], EPS_PS, float(np.float32(1.0 / QMAX)),
                mybir.AluOpType.max, mybir.AluOpType.mult)
            nc.vector.reciprocal(si[0:1, 1:2], si[0:1, 0:1])
            nc.vector.tensor_scalar(
                si[0:1, 2:3], si[0:1, 0:1], float(np.float32(2.0**-15)), None,
                mybir.AluOpType.mult)
            bc = stats.tile([P, 3], f32, tag="bc")
            nc.gpsimd.partition_broadcast(bc[:], si[0:1, :], channels=P)
            inv_t = bc[:, 1:2]
            scale_t = bc[:, 2:3]

            # ---------------- phase 2: quant + out = q @ Aw + b -----------
            with (
                tc.tile_pool(name="qpool", bufs=1) as qpool,
                tc.tile_pool(name="tpool", bufs=1) as tpool,
                tc.tile_pool(name="apool", bufs=2) as apool,
                tc.tile_pool(name="opool", bufs=4) as opool,
                tc.tile_pool(name="biasp", bufs=1) as biasp,
            ):
                # Phase-2 bulk loads would otherwise be released exactly at
                # mm1-end and their queue drain delays the tiny amax bounce
                # DMA. Gate them behind the bounce so they stream during the
                # collective wait instead.
                bias_t = biasp.tile([P, OUT_F], f32, tag="bias")
                bias_dma = nc.sync.dma_start(bias_t[:], bias_d[:, :])
                add_dep_helper(bias_dma.ins, bounce_dma.ins,
                               reason="stream during collective wait")

                # No explicit clip needed: scale = amax/QMAX with amax taken
                # over these same y values, so |round(y*inv)| <= QMAX always.
                q_t = qpool.tile([P, MR, TPC], bf16, tag="q")
                for mt in range(MT2):
                    sl = ts(mt, P)
                    krs = [(0, 4), (4, 8)] if mt == 0 else [(0, MR)]
                    for k0, k1 in krs:
                        t1 = tpool.tile([P, MR, P], f32, tag="t1")
                        # t1 = y*inv + MAGIC  (RNE in the f32 lattice)
                        nc.vector.tensor_scalar(
                            t1[:, k0:k1], y_t[:, k0:k1, sl], inv_t[:], MAGIC,
                            mybir.AluOpType.mult, mybir.AluOpType.add)
                        # y_q = (t1 - MAGIC) * scale -> bf16 (8 exact levels)
                        nc.vector.tensor_scalar(
                            q_t[:, k0:k1, sl], t1[:, k0:k1], -MAGIC, scale_t[:],
                            mybir.AluOpType.add, mybir.AluOpType.mult)

                for nt in range(NT2):
                    a_t = apool.tile([P, MR, 512], bf16, tag="aw")
                    a_dma = nc.sync.dma_start(a_t[:], aw_d[:, nt])
                    if nt <= 1:
                        add_dep_helper(a_dma.ins, bounce_dma.ins,
                                       reason="stream during collective wait")
                    for mt in range(MT2):
                        ps2 = psum.tile([P, 512], f32, tag="ps")
                        for kr in range(MR):
                            nc.tensor.matmul(
                                ps2[:], q_t[:, kr, ts(mt, P)], a_t[:, kr],
                                start=(kr == 0), stop=(kr == MR - 1))
                        o_t = opool.tile([P, 512], f32, tag="o")
                        nc.vector.tensor_tensor(
                            o_t[:], ps2[:], bias_t[:, ts(nt, 512)],
                            mybir.AluOpType.add)
                        nc.scalar.dma_start(out_d[ts(mt, P), ts(nt, 512)], o_t[:])

    nc.compile()
    return nc


def _get_nc():
    if "nc" not in _CACHE:
        _CACHE["nc"] = _build()
    return _CACHE["nc"]


def kernel(input, B_w, A_w, A_b):
    from concourse import bass_utils

    nc = _get_nc()

    f32 = np.float32
    f16 = np.float16
    bf16 = ml_dtypes.bfloat16
    e4 = ml_dtypes.float8_e4m3

    # Accept jax arrays / non-contiguous inputs
    input = np.asarray(input, dtype=f32)
    B_w = np.asarray(B_w, dtype=f32)
    A_w = np.asarray(A_w, dtype=f32)
    A_b = np.asarray(A_b, dtype=f32)

    # Weights, pre-scaled for the shared-psum 2^15 convention:
    #   main:  (Bh*2^8 fp16) @ (xh*2^7 fp16)
    #   corr:  (Bh*2^6 fp8) @ (xl*2^9 fp8)  +  (Bl*2^17 fp8) @ (x*2^-2 fp8)
    BT = np.ascontiguousarray(B_w.T).astype(f32) * np.float32(2.0**8)
    Bh16 = BT.astype(f16)                       # [IN_F, RANK] at 2^8
    Bl_s = BT - Bh16.astype(f32)
    w0 = (Bh16.astype(f32) * np.float32(0.25)).astype(e4)
    w1 = (Bl_s * np.float32(512.0)).astype(e4)
    bh = np.ascontiguousarray(
        Bh16.reshape(KT1, P, MR, P).transpose(1, 0, 2, 3))
    w0p = w0.reshape(KT1, P, MR, P).transpose(1, 0, 2, 3)
    w1p = w1.reshape(KT1, P, MR, P).transpose(1, 0, 2, 3)
    w8 = np.ascontiguousarray(np.stack([w0p, w1p], axis=3))  # [P,KT1,MR,2,P]

    AwT = np.ascontiguousarray(A_w.T)                        # [RANK, OUT_F]
    Aw = np.ascontiguousarray(
        AwT.astype(bf16).reshape(MR, P, NT2, 512).transpose(1, 2, 0, 3))

    bias_rep = np.ascontiguousarray(
        np.broadcast_to(A_b.astype(f32, copy=False), (P, OUT_F)))

    in_maps = []
    for c in range(NCORES):
        xT = np.ascontiguousarray(input[c * TPC : (c + 1) * TPC].T).astype(f32)
        xTs = xT * np.float32(2.0**7)
        xh16 = xTs.astype(f16)
        xl_s = xTs - xh16.astype(f32)
        m0 = (xl_s * np.float32(4.0)).astype(e4)
        m1 = (xTs * np.float32(2.0**-9)).astype(e4)
        xh = np.ascontiguousarray(
            xh16.reshape(KT1, P, NT1, 512).transpose(1, 2, 0, 3))
        m0p = m0.reshape(KT1, P, NT1, 512).transpose(1, 2, 0, 3)
        m1p = m1.reshape(KT1, P, NT1, 512).transpose(1, 2, 0, 3)
        m8 = np.ascontiguousarray(np.stack([m0p, m1p], axis=3))
        in_maps.append(
            {"xh": xh, "m8": m8, "bh": bh, "w8": w8, "aw": Aw, "bias": bias_rep}
        )

    res = bass_utils.run_bass_kernel_spmd(nc, in_maps, core_ids=list(range(NCORES)))
    out = np.concatenate([res.results[c]["out"] for c in range(NCORES)], axis=0)
    return out.astype(np.float32, copy=False)


# revision 9
# speedup vs baseline: 1.3348x; 1.0213x over previous
"""Trainium2 distributed kernel for ALRDLinear + 3-bit per-tensor fake-quant.

Reference computation (tokens=8192, in=4096, rank=1024, out=4096, f32):
    y   = input @ B_w.T                       # [tokens, rank]
    y_q = fake_quant(y)                       # per-tensor symmetric 3-bit
    out = y_q @ A_w.T + A_b                   # [tokens, out]

Distribution: data-parallel over tokens across 8 NeuronCores (1024 tok/core).
Weights replicated. The only cross-core dependency is the per-tensor amax,
exchanged with one 64-byte AllGather and reduced locally.

Numerics: y needs ~fp16x2 precision because it feeds round(y/scale) and
rounding-boundary flips are amplified by the 3-bit step (bf16 or fp32r
matmuls fail the 2e-2 gate). matmul1 = one fp16 main pass (Bh@xh) plus ONE
fp8-DoubleRow pass computing BOTH correction terms fused: each PE cell holds
the weight pair [Bh8, Bl8] and streams the pair [xl8, x8], accumulating
Bh@xl + Bl@x at 2x contraction rate (K=256/instruction). All operands are
pre-scaled so every product lands in the same PSUM at 2^15 scale; the 2^-15
is folded into the scalar scale/inv computation after the amax AllGather.
Measured y error std ~1e-5 -> final rel err ~3.5e-3 (gate 2e-2).

Quantization uses the +1.5*2^23 RNE trick with no clip (|q| <= 3 by
construction of scale); y_q = q*scale stored bf16. Matmul2 runs y_q (bf16)
against bf16 A-weights, bias added on eviction.

Perf notes (measured on TRN2): PE issues N=512 matmuls every ~263ns at the
collective-capped 2.08GHz clock; fp8 DoubleRow issues at the same per-MM
rate (2x work). Loop order is k-outer/mr-inner for the fp16 sweep and
mr-outer/k-inner for the DR sweep so each streamed x-chunk is consumed by
all 8 rank tiles immediately (~180GB/s just-in-time demand instead of the
~475GB/s a mr-outer fp16 sweep would need, which starved the PE for ~30us).
B-weights are re-streamed per token-tile (extra 16MB, free bandwidth-wise)
so SBUF holds both token-tiles of x. A dummy AllGather issued at kernel
start warms the collective path/absorbs core launch skew off the critical
path; the real 64B amax AllGather then completes in ~10-20us instead of
~29us. Phase-2 bulk weight/bias loads are gated on the amax bounce DMA so
they stream through the collective-wait window.
"""

import numpy as np
import ml_dtypes

P = 128
TOK, IN_F, OUT_F, RANK = 8192, 4096, 4096, 1024
NCORES = 8
TPC = TOK // NCORES            # tokens per core
KT1 = IN_F // P                # 32 contraction tiles for matmul1
MR = RANK // P                 # 8 rank tiles
NT1 = TPC // 512               # 2 token column-tiles in matmul1
MT2 = TPC // P                 # 8 token row-tiles in matmul2
NT2 = OUT_F // 512             # 8 out-feature tiles

QMAX = 3.0
MAGIC = 1.5 * 2.0**23          # round-to-nearest-even integer trick
EPS_PS = 1e-8 * 2.0**15        # reference's 1e-8 floor, at psum scale

_CACHE = {}


def _build():
    import concourse.mybir as mybir
    import concourse.tile as tile
    from concourse import bacc
    from concourse import bass_isa

    nc = bacc.Bacc(None, target_bir_lowering=False, debug=False, num_devices=NCORES)
    f32, f16, bf16 = mybir.dt.float32, mybir.dt.float16, mybir.dt.bfloat16
    f8 = mybir.dt.float8e4
    DR = mybir.MatmulPerfMode.DoubleRow

    xh_d = nc.dram_tensor("xh", [P, NT1, KT1, 512], f16, kind="ExternalInput")
    m8_d = nc.dram_tensor("m8", [P, NT1, KT1, 2, 512], f8, kind="ExternalInput")
    bh_d = nc.dram_tensor("bh", [P, KT1, MR, P], f16, kind="ExternalInput")
    w8_d = nc.dram_tensor("w8", [P, KT1, MR, 2, P], f8, kind="ExternalInput")
    aw_d = nc.dram_tensor("aw", [P, NT2, MR, 512], bf16, kind="ExternalInput")
    bias_d = nc.dram_tensor("bias", [P, OUT_F], f32, kind="ExternalInput")
    out_d = nc.dram_tensor("out", [TPC, OUT_F], f32, kind="ExternalOutput")

    cc_in = nc.dram_tensor("cc_in", [1, 16], f32)
    cc_out = nc.dram_tensor("cc_out", [NCORES, 16], f32, addr_space="Shared")
    cc_ind = nc.dram_tensor("cc_ind", [1, 16], f32)
    cc_outd = nc.dram_tensor("cc_outd", [NCORES, 16], f32, addr_space="Shared")

    ts = lambda i, s: slice(i * s, (i + 1) * s)

    from concourse.tile_rust import add_dep_helper

    with tile.TileContext(nc) as tc:
        with (
            tc.tile_pool(name="stats", bufs=1) as stats,
            tc.tile_pool(name="ypool", bufs=1) as ypool,
            tc.tile_pool(name="psum", bufs=8, space="PSUM") as psum,
        ):
            y_t = ypool.tile([P, MR, TPC], f32, tag="y")
            am_part = stats.tile([P, MR * NT1], f32, tag="am_part")
            am1 = stats.tile([P, 1], f32, tag="am1")
            am_b = stats.tile([P, 1], f32, tag="am_b")

            # Dummy AllGather: warms the collective path (ring setup, core
            # launch-skew absorption) while the PE crunches matmul1. No
            # return DMA -- nothing consumes it; the real collective queues
            # behind it on the CC stream.
            drow = stats.tile([1, 16], f32, tag="drow")
            nc.vector.memset(drow[0:1, :], 1.0)
            nc.sync.dma_start(cc_ind[:, :], drow[0:1, :])
            cc_dummy = nc.gpsimd.collective_compute(
                "AllGather", mybir.AluOpType.bypass,
                replica_groups=[list(range(NCORES))],
                ins=[cc_ind.ap().opt()], outs=[cc_outd.ap().opt()])

            # ---------------- phase 1: y.T = B @ x.T ---------------------
            # per nt: sweep1 = fp16 main, sweep2 = fp8 DoubleRow corrections.
            # Both sweeps run k-outer/mr-inner so each just-in-time x chunk
            # is consumed by all 8 rank tiles immediately (~180GB/s demand);
            # sweep2's last KTAIL k-tiles run mr-outer so the 8 psum tiles
            # close staggered and their evictions hide under PE work.
            KG = 4       # k-tiles per streamed weight/x chunk
            NG = KT1 // KG
            KTAIL = 4    # k-tiles in the staggered-eviction tail
            KB = KT1 - KTAIL
            with (
                tc.tile_pool(name="xpool", bufs=1) as xpool,
                tc.tile_pool(name="bpool", bufs=2) as bpool,
                tc.tile_pool(name="wpool", bufs=2) as wpool,
                tc.tile_pool(name="wtpool", bufs=1) as wtpool,
            ):
                xh_t = xpool.tile([P, NT1, KT1, 512], f16, tag="xh")
                m8_t = xpool.tile([P, NT1, KT1, 2, 512], f8, tag="m8")
                w8tail_t = wtpool.tile([P, KTAIL, MR, 2, P], f8, tag="w8tail")

                for nt in range(NT1):
                    ps = []
                    for mr in range(MR):
                        pst = psum.tile([P, 512], f32, tag="ps",
                                        name=f"ps_{nt}_{mr}")
                        ps.append(pst)
                    # sweep1: fp16 main pass (k-outer); weight groups are
                    # 1MB double-buffered transfers, x chunks lead by 2.
                    # The very first group is emitted at k granularity so the
                    # first matmul only waits for 384KB, not 1.5MB.
                    first = (nt == 0)
                    if not first:
                        nc.sync.dma_start(
                            xh_t[:, nt, ts(0, KG)], xh_d[:, nt, ts(0, KG)])
                        nc.sync.dma_start(
                            xh_t[:, nt, ts(1, KG)], xh_d[:, nt, ts(1, KG)])
                    for g in range(NG):
                        bh_t = bpool.tile([P, KG, MR, P], f16, tag="bh")
                        if first and g == 0:
                            # k-granular interleave: first matmul waits for
                            # just xh[k0]+bh[k0] (384KB), not a full group
                            for k in range(KG):
                                nc.sync.dma_start(
                                    xh_t[:, nt, k : k + 1], xh_d[:, nt, k : k + 1])
                                nc.sync.dma_start(
                                    bh_t[:, k : k + 1], bh_d[:, k : k + 1])
                            nc.sync.dma_start(
                                xh_t[:, nt, ts(1, KG)], xh_d[:, nt, ts(1, KG)])
                        else:
                            nc.sync.dma_start(bh_t[:], bh_d[:, ts(g, KG)])
                        if g + 2 < NG:
                            nc.sync.dma_start(
                                xh_t[:, nt, ts(g + 2, KG)],
                                xh_d[:, nt, ts(g + 2, KG)])
                        if nt == 0 and g in (4, 5):
                            j = g - 4
                            h = KTAIL // 2
                            nc.sync.dma_start(
                                w8tail_t[:, ts(j, h)], w8_d[:, KB + j * h : KB + (j + 1) * h])
                        for kk in range(KG):
                            k = g * KG + kk
                            for mr in range(MR):
                                nc.tensor.matmul(
                                    ps[mr][:], bh_t[:, kk, mr], xh_t[:, nt, k],
                                    start=(k == 0), stop=False)
                    # sweep2 head: DR corrections, k-outer over k < KB
                    nc.sync.dma_start(m8_t[:, nt, ts(0, KG)], m8_d[:, nt, ts(0, KG)])
                    nc.sync.dma_start(m8_t[:, nt, ts(1, KG)], m8_d[:, nt, ts(1, KG)])
                    for g in range(KB // KG):
                        if g + 2 < NG:
                            nc.sync.dma_start(
                                m8_t[:, nt, ts(g + 2, KG)],
                                m8_d[:, nt, ts(g + 2, KG)])
                        w8_t = wpool.tile([P, KG, MR, 2, P], f8, tag="w8")
                        nc.sync.dma_start(w8_t[:], w8_d[:, ts(g, KG)])
                        for kk in range(KG):
                            k = g * KG + kk
                            for mr in range(MR):
                                nc.tensor.matmul(
                                    ps[mr][:], w8_t[:, kk, mr], m8_t[:, nt, k],
                                    start=False, stop=False, perf_mode=DR)
                    # sweep2 tail: mr-outer, staggered psum close + eviction
                    for mr in range(MR):
                        for k in range(KB, KT1):
                            nc.tensor.matmul(
                                ps[mr][:], w8tail_t[:, k - KB, mr],
                                m8_t[:, nt, k],
                                start=False, stop=(k == KT1 - 1),
                                perf_mode=DR)
                        idx = nt * MR + mr
                        nc.vector.tensor_reduce(
                            am_part[:, idx : idx + 1], ps[mr][:],
                            axis=mybir.AxisListType.X, op=mybir.AluOpType.max,
                            apply_absolute_value=True)
                        nc.scalar.copy(y_t[:, mr, ts(nt, 512)], ps[mr][:])

            # ---------------- amax all-gather + scale ---------------------
            # Keep every cross-core transfer a single contiguous descriptor
            # (a [128,1] partition-strided DMA costs ~7.5us in 4B descriptors).
            nc.vector.tensor_reduce(
                am1[:], am_part[:], axis=mybir.AxisListType.X,
                op=mybir.AluOpType.max)
            nc.gpsimd.partition_all_reduce(
                am_b[:], am1[:], channels=P, reduce_op=bass_isa.ReduceOp.max)
            row16 = stats.tile([1, 16], f32, tag="row16")
            nc.vector.tensor_copy(row16[0:1, :], am_b[0:1, 0:1].to_broadcast([1, 16]))
            bounce_dma = nc.sync.dma_start(cc_in[:, :], row16[0:1, :])
            cc_inst = nc.gpsimd.collective_compute(
                "AllGather", mybir.AluOpType.bypass,
                replica_groups=[list(range(NCORES))],
                ins=[cc_in.ap().opt()], outs=[cc_out.ap().opt()])
            add_dep_helper(cc_inst.ins, cc_dummy.ins,
                           reason="real collective after warmup dummy")
            amrow = stats.tile([1, NCORES * 16], f32, tag="amrow")
            ret_dma = nc.sync.dma_start(
                amrow[0:1, :], cc_out[:, :].rearrange("c x -> (c x)")[None, :])
            amg1 = stats.tile([1, 1], f32, tag="amg1")
            nc.vector.tensor_reduce(
                amg1[0:1, :], amrow[0:1, :], axis=mybir.AxisListType.X,
                op=mybir.AluOpType.max)
            # si = [u, 1/u, u*2^-15] with u = max(amax_ps, eps)/QMAX:
            # 1/u is the quant multiplier for psum-scaled y; u*2^-15 is the
            # true dequant scale.
            si = stats.tile([1, 3], f32, tag="si")
            nc.vector.tensor_scalar(
                si[0:1, 0:1], amg1[0:1, :], EPS_PS, float(np.float32(1.0 / QMAX)),
                mybir.AluOpType.max, mybir.AluOpType.mult)
            nc.vector.reciprocal(si[0:1, 1:2], si[0:1, 0:1])
            nc.vector.tensor_scalar(
                si[0:1, 2:3], si[0:1, 0:1], float(np.float32(2.0**-15)), None,
                mybir.AluOpType.mult)
            bc = stats.tile([P, 3], f32, tag="bc")
            nc.gpsimd.partition_broadcast(bc[:], si[0:1, :], channels=P)
            inv_t = bc[:, 1:2]
            scale_t = bc[:, 2:3]

            # ---------------- phase 2: quant + out = q @ Aw + b -----------
            with (
                tc.tile_pool(name="qpool", bufs=1) as qpool,
                tc.tile_pool(name="tpool", bufs=1) as tpool,
                tc.tile_pool(name="apool", bufs=2) as apool,
                tc.tile_pool(name="opool", bufs=4) as opool,
                tc.tile_pool(name="biasp", bufs=1) as biasp,
            ):
                # Phase-2 bulk loads would otherwise be released exactly at
                # mm1-end and their queue drain delays the tiny amax bounce
                # DMA. Gate them behind the bounce so they stream during the
                # collective wait instead.
                bias_t = biasp.tile([P, OUT_F], f32, tag="bias")
                bias_dma = nc.sync.dma_start(bias_t[:], bias_d[:, :])
                add_dep_helper(bias_dma.ins, bounce_dma.ins,
                               reason="stream during collective wait")

                # No explicit clip needed: scale = amax/QMAX with amax taken
                # over these same y values, so |round(y*inv)| <= QMAX always.
                q_t = qpool.tile([P, MR, TPC], bf16, tag="q")
                for mt in range(MT2):
                    sl = ts(mt, P)
                    krs = [(0, 4), (4, 8)] if mt == 0 else [(0, MR)]
                    for k0, k1 in krs:
                        t1 = tpool.tile([P, MR, P], f32, tag="t1")
                        # t1 = y*inv + MAGIC  (RNE in the f32 lattice)
                        nc.vector.tensor_scalar(
                            t1[:, k0:k1], y_t[:, k0:k1, sl], inv_t[:], MAGIC,
                            mybir.AluOpType.mult, mybir.AluOpType.add)
                        # y_q = (t1 - MAGIC) * scale -> bf16 (8 exact levels)
                        nc.vector.tensor_scalar(
                            q_t[:, k0:k1, sl], t1[:, k0:k1], -MAGIC, scale_t[:],
                            mybir.AluOpType.add, mybir.AluOpType.mult)

                for nt in range(NT2):
                    a_t = apool.tile([P, MR, 512], bf16, tag="aw")
                    a_dma = nc.sync.dma_start(a_t[:], aw_d[:, nt])
                    if nt <= 1:
                        add_dep_helper(a_dma.ins, bounce_dma.ins,
                                       reason="stream during collective wait")
                    for mt in range(MT2):
                        ps2 = psum.tile([P, 512], f32, tag="ps")
                        for kr in range(MR):
                            nc.tensor.matmul(
                                ps2[:], q_t[:, kr, ts(mt, P)], a_t[:, kr],
                                start=(kr == 0), stop=(kr == MR - 1))
                        o_t = opool.tile([P, 512], f32, tag="o")
                        nc.vector.tensor_tensor(
                            o_t[:], ps2[:], bias_t[:, ts(nt, 512)],
                            mybir.AluOpType.add)
                        # alternate trigger engines -> two DMA queues drain
                        # the 16MB output in parallel (one queue saturates at
                        # ~122GB/s vs the 118GB/s demand and drains late)
                        eng = nc.scalar if mt % 2 == 0 else nc.sync
                        eng.dma_start(out_d[ts(mt, P), ts(nt, 512)], o_t[:])

    nc.compile()
    return nc


def _get_nc():
    if "nc" not in _CACHE:
        _CACHE["nc"] = _build()
    return _CACHE["nc"]


def kernel(input, B_w, A_w, A_b):
    from concourse import bass_utils

    nc = _get_nc()

    f32 = np.float32
    f16 = np.float16
    bf16 = ml_dtypes.bfloat16
    e4 = ml_dtypes.float8_e4m3

    # Accept jax arrays / non-contiguous inputs
    input = np.asarray(input, dtype=f32)
    B_w = np.asarray(B_w, dtype=f32)
    A_w = np.asarray(A_w, dtype=f32)
    A_b = np.asarray(A_b, dtype=f32)

    # Weights, pre-scaled for the shared-psum 2^15 convention:
    #   main:  (Bh*2^8 fp16) @ (xh*2^7 fp16)
    #   corr:  (Bh*2^6 fp8) @ (xl*2^9 fp8)  +  (Bl*2^17 fp8) @ (x*2^-2 fp8)
    BT = np.ascontiguousarray(B_w.T).astype(f32) * np.float32(2.0**8)
    Bh16 = BT.astype(f16)                       # [IN_F, RANK] at 2^8
    Bl_s = BT - Bh16.astype(f32)
    w0 = (Bh16.astype(f32) * np.float32(0.25)).astype(e4)
    w1 = (Bl_s * np.float32(512.0)).astype(e4)
    bh = np.ascontiguousarray(
        Bh16.reshape(KT1, P, MR, P).transpose(1, 0, 2, 3))
    w0p = w0.reshape(KT1, P, MR, P).transpose(1, 0, 2, 3)
    w1p = w1.reshape(KT1, P, MR, P).transpose(1, 0, 2, 3)
    w8 = np.ascontiguousarray(np.stack([w0p, w1p], axis=3))  # [P,KT1,MR,2,P]

    AwT = np.ascontiguousarray(A_w.T)                        # [RANK, OUT_F]
    Aw = np.ascontiguousarray(
        AwT.astype(bf16).reshape(MR, P, NT2, 512).transpose(1, 2, 0, 3))

    bias_rep = np.ascontiguousarray(
        np.broadcast_to(A_b.astype(f32, copy=False), (P, OUT_F)))

    in_maps = []
    for c in range(NCORES):
        xT = np.ascontiguousarray(input[c * TPC : (c + 1) * TPC].T).astype(f32)
        xTs = xT * np.float32(2.0**7)
        xh16 = xTs.astype(f16)
        xl_s = xTs - xh16.astype(f32)
        m0 = (xl_s * np.float32(4.0)).astype(e4)
        m1 = (xTs * np.float32(2.0**-9)).astype(e4)
        xh = np.ascontiguousarray(
            xh16.reshape(KT1, P, NT1, 512).transpose(1, 2, 0, 3))
        m0p = m0.reshape(KT1, P, NT1, 512).transpose(1, 2, 0, 3)
        m1p = m1.reshape(KT1, P, NT1, 512).transpose(1, 2, 0, 3)
        m8 = np.ascontiguousarray(np.stack([m0p, m1p], axis=3))
        in_maps.append(
            {"xh": xh, "m8": m8, "bh": bh, "w8": w8, "aw": Aw, "bias": bias_rep}
        )

    res = bass_utils.run_bass_kernel_spmd(nc, in_maps, core_ids=list(range(NCORES)))
    out = np.concatenate([res.results[c]["out"] for c in range(NCORES)], axis=0)
    return out.astype(np.float32, copy=False)


# revision 11
# speedup vs baseline: 1.3417x; 1.0052x over previous
"""Trainium2 distributed kernel for ALRDLinear + 3-bit per-tensor fake-quant.

Reference computation (tokens=8192, in=4096, rank=1024, out=4096, f32):
    y   = input @ B_w.T                       # [tokens, rank]
    y_q = fake_quant(y)                       # per-tensor symmetric 3-bit
    out = y_q @ A_w.T + A_b                   # [tokens, out]

Distribution: data-parallel over tokens across 8 NeuronCores (1024 tok/core).
Weights replicated. The only cross-core dependency is the per-tensor amax,
exchanged with one 64-byte AllGather and reduced locally.

Numerics: y needs ~fp16x2 precision because it feeds round(y/scale) and
rounding-boundary flips are amplified by the 3-bit step (bf16 or fp32r
matmuls fail the 2e-2 gate). matmul1 = one fp16 main pass (Bh@xh) plus ONE
fp8-DoubleRow pass computing BOTH correction terms fused: each PE cell holds
the weight pair [Bh8, Bl8] and streams the pair [xl8, x8], accumulating
Bh@xl + Bl@x at 2x contraction rate (K=256/instruction). All operands are
pre-scaled so every product lands in the same PSUM at 2^15 scale; the 2^-15
is folded into the scalar scale/inv computation after the amax AllGather.
Measured y error std ~1e-5 -> final rel err ~3.5e-3 (gate 2e-2).

Quantization uses the +1.5*2^23 RNE trick with no clip (|q| <= 3 by
construction of scale); y_q = q*scale stored bf16. Matmul2 runs y_q (bf16)
against bf16 A-weights, bias added on eviction.

Perf notes (measured on TRN2): PE issues N=512 matmuls every ~263ns at the
collective-capped 2.08GHz clock; fp8 DoubleRow issues at the same per-MM
rate (2x work). Loop order is k-outer/mr-inner for the fp16 sweep and
mr-outer/k-inner for the DR sweep so each streamed x-chunk is consumed by
all 8 rank tiles immediately (~180GB/s just-in-time demand instead of the
~475GB/s a mr-outer fp16 sweep would need, which starved the PE for ~30us).
B-weights are re-streamed per token-tile (extra 16MB, free bandwidth-wise)
so SBUF holds both token-tiles of x. A dummy AllGather issued at kernel
start warms the collective path/absorbs core launch skew off the critical
path; the real 64B amax AllGather then completes in ~10-20us instead of
~29us. Phase-2 bulk weight/bias loads are gated on the amax bounce DMA so
they stream through the collective-wait window.
"""

import numpy as np
import ml_dtypes

P = 128
TOK, IN_F, OUT_F, RANK = 8192, 4096, 4096, 1024
NCORES = 8
TPC = TOK // NCORES            # tokens per core
KT1 = IN_F // P                # 32 contraction tiles for matmul1
MR = RANK // P                 # 8 rank tiles
NT1 = TPC // 512               # 2 token column-tiles in matmul1
MT2 = TPC // P                 # 8 token row-tiles in matmul2
NT2 = OUT_F // 512             # 8 out-feature tiles

QMAX = 3.0
MAGIC = 1.5 * 2.0**23          # round-to-nearest-even integer trick
EPS_PS = 1e-8 * 2.0**15        # reference's 1e-8 floor, at psum scale

_CACHE = {}


def _build():
    import concourse.mybir as mybir
    import concourse.tile as tile
    from concourse import bacc
    from concourse import bass_isa

    nc = bacc.Bacc(None, target_bir_lowering=False, debug=False, num_devices=NCORES)
    f32, f16, bf16 = mybir.dt.float32, mybir.dt.float16, mybir.dt.bfloat16
    f8 = mybir.dt.float8e4
    DR = mybir.MatmulPerfMode.DoubleRow

    xh_d = nc.dram_tensor("xh", [P, NT1, KT1, 512], f16, kind="ExternalInput")
    m8_d = nc.dram_tensor("m8", [P, NT1, KT1, 2, 512], f8, kind="ExternalInput")
    bh_d = nc.dram_tensor("bh", [P, KT1, MR, P], f16, kind="ExternalInput")
    w8_d = nc.dram_tensor("w8", [P, KT1, MR, 2, P], f8, kind="ExternalInput")
    aw_d = nc.dram_tensor("aw", [P, NT2, MR, 512], bf16, kind="ExternalInput")
    bias_d = nc.dram_tensor("bias", [P, OUT_F], f32, kind="ExternalInput")
    out_d = nc.dram_tensor("out", [TPC, OUT_F], f32, kind="ExternalOutput")

    cc_in = nc.dram_tensor("cc_in", [1, 16], f32)
    cc_out = nc.dram_tensor("cc_out", [NCORES, 16], f32, addr_space="Shared")
    cc_ind = nc.dram_tensor("cc_ind", [1, 16], f32)
    cc_outd = nc.dram_tensor("cc_outd", [NCORES, 16], f32, addr_space="Shared")

    ts = lambda i, s: slice(i * s, (i + 1) * s)

    from concourse.tile_rust import add_dep_helper

    with tile.TileContext(nc) as tc:
        with (
            tc.tile_pool(name="stats", bufs=1) as stats,
            tc.tile_pool(name="ypool", bufs=1) as ypool,
            tc.tile_pool(name="psum", bufs=8, space="PSUM") as psum,
        ):
            y_t = ypool.tile([P, MR, TPC], f32, tag="y")
            am_part = stats.tile([P, MR * NT1], f32, tag="am_part")
            am1 = stats.tile([P, 1], f32, tag="am1")
            am_b = stats.tile([P, 1], f32, tag="am_b")

            # Dummy AllGather: warms the collective path (ring setup, core
            # launch-skew absorption) while the PE crunches matmul1. No
            # return DMA -- nothing consumes it; the real collective queues
            # behind it on the CC stream.
            drow = stats.tile([1, 16], f32, tag="drow")
            nc.vector.memset(drow[0:1, :], 1.0)
            nc.sync.dma_start(cc_ind[:, :], drow[0:1, :])
            cc_dummy = nc.gpsimd.collective_compute(
                "AllGather", mybir.AluOpType.bypass,
                replica_groups=[list(range(NCORES))],
                ins=[cc_ind.ap().opt()], outs=[cc_outd.ap().opt()])

            # ---------------- phase 1: y.T = B @ x.T ---------------------
            # per nt: sweep1 = fp16 main, sweep2 = fp8 DoubleRow corrections.
            # Both sweeps run k-outer/mr-inner so each just-in-time x chunk
            # is consumed by all 8 rank tiles immediately (~180GB/s demand);
            # sweep2's last KTAIL k-tiles run mr-outer so the 8 psum tiles
            # close staggered and their evictions hide under PE work.
            KG = 4       # k-tiles per streamed weight/x chunk
            NG = KT1 // KG
            KTAIL = 4    # k-tiles in the staggered-eviction tail
            KB = KT1 - KTAIL
            with (
                tc.tile_pool(name="xpool", bufs=1) as xpool,
                tc.tile_pool(name="bpool", bufs=2) as bpool,
                tc.tile_pool(name="wpool", bufs=2) as wpool,
                tc.tile_pool(name="wtpool", bufs=1) as wtpool,
            ):
                xh_t = xpool.tile([P, NT1, KT1, 512], f16, tag="xh")
                m8_t = xpool.tile([P, NT1, KT1, 2, 512], f8, tag="m8")
                w8tail_t = wtpool.tile([P, KTAIL, MR, 2, P], f8, tag="w8tail")

                for nt in range(NT1):
                    ps = []
                    for mr in range(MR):
                        pst = psum.tile([P, 512], f32, tag="ps",
                                        name=f"ps_{nt}_{mr}")
                        ps.append(pst)
                    # sweep1: fp16 main pass (k-outer); weight groups are
                    # 1MB double-buffered transfers, x chunks lead by 2.
                    # The very first group is emitted at k granularity so the
                    # first matmul only waits for 384KB, not 1.5MB.
                    # nt0 runs x-lookahead 1 (strict need-order while the
                    # transfer queue fills); nt1 gets lookahead 2 (its x
                    # prefetches during nt0's sweep2)
                    first = (nt == 0)
                    lk = 1 if first else 2
                    if not first:
                        for gg in range(lk):
                            nc.sync.dma_start(
                                xh_t[:, nt, ts(gg, KG)], xh_d[:, nt, ts(gg, KG)])
                    for g in range(NG):
                        bh_t = bpool.tile([P, KG, MR, P], f16, tag="bh")
                        if first and g == 0:
                            # k-granular interleave: first matmul waits for
                            # just xh[k0]+bh[k0] (384KB), not a full group
                            for k in range(KG):
                                nc.sync.dma_start(
                                    xh_t[:, nt, k : k + 1], xh_d[:, nt, k : k + 1])
                                nc.sync.dma_start(
                                    bh_t[:, k : k + 1], bh_d[:, k : k + 1])
                        else:
                            nc.sync.dma_start(bh_t[:], bh_d[:, ts(g, KG)])
                        if g + lk < NG:
                            nc.sync.dma_start(
                                xh_t[:, nt, ts(g + lk, KG)],
                                xh_d[:, nt, ts(g + lk, KG)])
                        if nt == 0 and g in (4, 5):
                            j = g - 4
                            h = KTAIL // 2
                            nc.sync.dma_start(
                                w8tail_t[:, ts(j, h)], w8_d[:, KB + j * h : KB + (j + 1) * h])
                        for kk in range(KG):
                            k = g * KG + kk
                            for mr in range(MR):
                                nc.tensor.matmul(
                                    ps[mr][:], bh_t[:, kk, mr], xh_t[:, nt, k],
                                    start=(k == 0), stop=False)
                    # sweep2 head: DR corrections, k-outer over k < KB
                    nc.sync.dma_start(m8_t[:, nt, ts(0, KG)], m8_d[:, nt, ts(0, KG)])
                    nc.sync.dma_start(m8_t[:, nt, ts(1, KG)], m8_d[:, nt, ts(1, KG)])
                    for g in range(KB // KG):
                        if g + 2 < NG:
                            nc.sync.dma_start(
                                m8_t[:, nt, ts(g + 2, KG)],
                                m8_d[:, nt, ts(g + 2, KG)])
                        w8_t = wpool.tile([P, KG, MR, 2, P], f8, tag="w8")
                        nc.sync.dma_start(w8_t[:], w8_d[:, ts(g, KG)])
                        for kk in range(KG):
                            k = g * KG + kk
                            for mr in range(MR):
                                nc.tensor.matmul(
                                    ps[mr][:], w8_t[:, kk, mr], m8_t[:, nt, k],
                                    start=False, stop=False, perf_mode=DR)
                    # sweep2 tail: mr-outer, staggered psum close + eviction
                    for mr in range(MR):
                        for k in range(KB, KT1):
                            nc.tensor.matmul(
                                ps[mr][:], w8tail_t[:, k - KB, mr],
                                m8_t[:, nt, k],
                                start=False, stop=(k == KT1 - 1),
                                perf_mode=DR)
                        idx = nt * MR + mr
                        nc.vector.tensor_reduce(
                            am_part[:, idx : idx + 1], ps[mr][:],
                            axis=mybir.AxisListType.X, op=mybir.AluOpType.max,
                            apply_absolute_value=True)
                        nc.scalar.copy(y_t[:, mr, ts(nt, 512)], ps[mr][:])

            # ---------------- amax all-gather + scale ---------------------
            # Keep every cross-core transfer a single contiguous descriptor
            # (a [128,1] partition-strided DMA costs ~7.5us in 4B descriptors).
            nc.vector.tensor_reduce(
                am1[:], am_part[:], axis=mybir.AxisListType.X,
                op=mybir.AluOpType.max)
            nc.gpsimd.partition_all_reduce(
                am_b[:], am1[:], channels=P, reduce_op=bass_isa.ReduceOp.max)
            row16 = stats.tile([1, 16], f32, tag="row16")
            nc.vector.tensor_copy(row16[0:1, :], am_b[0:1, 0:1].to_broadcast([1, 16]))
            bounce_dma = nc.sync.dma_start(cc_in[:, :], row16[0:1, :])
            cc_inst = nc.gpsimd.collective_compute(
                "AllGather", mybir.AluOpType.bypass,
                replica_groups=[list(range(NCORES))],
                ins=[cc_in.ap().opt()], outs=[cc_out.ap().opt()])
            add_dep_helper(cc_inst.ins, cc_dummy.ins,
                           reason="real collective after warmup dummy")
            amrow = stats.tile([1, NCORES * 16], f32, tag="amrow")
            ret_dma = nc.sync.dma_start(
                amrow[0:1, :], cc_out[:, :].rearrange("c x -> (c x)")[None, :])
            amg1 = stats.tile([1, 1], f32, tag="amg1")
            nc.vector.tensor_reduce(
                amg1[0:1, :], amrow[0:1, :], axis=mybir.AxisListType.X,
                op=mybir.AluOpType.max)
            # si = [u, 1/u, u*2^-15] with u = max(amax_ps, eps)/QMAX:
            # 1/u is the quant multiplier for psum-scaled y; u*2^-15 is the
            # true dequant scale.
            si = stats.tile([1, 3], f32, tag="si")
            nc.vector.tensor_scalar(
                si[0:1, 0:1], amg1[0:1, :], EPS_PS, float(np.float32(1.0 / QMAX)),
                mybir.AluOpType.max, mybir.AluOpType.mult)
            nc.vector.reciprocal(si[0:1, 1:2], si[0:1, 0:1])
            nc.vector.tensor_scalar(
                si[0:1, 2:3], si[0:1, 0:1], float(np.float32(2.0**-15)), None,
                mybir.AluOpType.mult)
            bc = stats.tile([P, 3], f32, tag="bc")
            nc.gpsimd.partition_broadcast(bc[:], si[0:1, :], channels=P)
            inv_t = bc[:, 1:2]
            scale_t = bc[:, 2:3]

            # ---------------- phase 2: quant + out = q @ Aw + b -----------
            with (
                tc.tile_pool(name="qpool", bufs=1) as qpool,
                tc.tile_pool(name="tpool", bufs=1) as tpool,
                tc.tile_pool(name="apool", bufs=2) as apool,
                tc.tile_pool(name="opool", bufs=4) as opool,
                tc.tile_pool(name="biasp", bufs=1) as biasp,
            ):
                # Phase-2 bulk loads would otherwise be released exactly at
                # mm1-end and their queue drain delays the tiny amax bounce
                # DMA. Gate them behind the bounce so they stream during the
                # collective wait instead.
                bias_t = biasp.tile([P, OUT_F], f32, tag="bias")
                bias_dma = nc.sync.dma_start(bias_t[:], bias_d[:, :])
                add_dep_helper(bias_dma.ins, bounce_dma.ins,
                               reason="stream during collective wait")

                # No explicit clip needed: scale = amax/QMAX with amax taken
                # over these same y values, so |round(y*inv)| <= QMAX always.
                q_t = qpool.tile([P, MR, TPC], bf16, tag="q")
                for mt in range(MT2):
                    sl = ts(mt, P)
                    krs = ([(0, 2), (2, 4), (4, 6), (6, 8)] if mt == 0
                           else [(0, MR)])
                    for k0, k1 in krs:
                        t1 = tpool.tile([P, MR, P], f32, tag="t1")
                        # t1 = y*inv + MAGIC  (RNE in the f32 lattice)
                        nc.vector.tensor_scalar(
                            t1[:, k0:k1], y_t[:, k0:k1, sl], inv_t[:], MAGIC,
                            mybir.AluOpType.mult, mybir.AluOpType.add)
                        # y_q = (t1 - MAGIC) * scale -> bf16 (8 exact levels)
                        nc.vector.tensor_scalar(
                            q_t[:, k0:k1, sl], t1[:, k0:k1], -MAGIC, scale_t[:],
                            mybir.AluOpType.add, mybir.AluOpType.mult)

                for nt in range(NT2):
                    a_t = apool.tile([P, MR, 512], bf16, tag="aw")
                    a_dma = nc.sync.dma_start(a_t[:], aw_d[:, nt])
                    if nt <= 1:
                        add_dep_helper(a_dma.ins, bounce_dma.ins,
                                       reason="stream during collective wait")
                    for mt in range(MT2):
                        ps2 = psum.tile([P, 512], f32, tag="ps")
                        for kr in range(MR):
                            nc.tensor.matmul(
                                ps2[:], q_t[:, kr, ts(mt, P)], a_t[:, kr],
                                start=(kr == 0), stop=(kr == MR - 1))
                        o_t = opool.tile([P, 512], f32, tag="o")
                        nc.vector.tensor_tensor(
                            o_t[:], ps2[:], bias_t[:, ts(nt, 512)],
                            mybir.AluOpType.add)
                        # alternate trigger engines -> two DMA queues drain
                        # the 16MB output in parallel (one queue saturates at
                        # ~122GB/s vs the 118GB/s demand and drains late)
                        eng = nc.scalar if mt % 2 == 0 else nc.sync
                        eng.dma_start(out_d[ts(mt, P), ts(nt, 512)], o_t[:])

    nc.compile()
    return nc


def _get_nc():
    if "nc" not in _CACHE:
        _CACHE["nc"] = _build()
    return _CACHE["nc"]


def kernel(input, B_w, A_w, A_b):
    from concourse import bass_utils

    nc = _get_nc()

    f32 = np.float32
    f16 = np.float16
    bf16 = ml_dtypes.bfloat16
    e4 = ml_dtypes.float8_e4m3

    # Accept jax arrays / non-contiguous inputs
    input = np.asarray(input, dtype=f32)
    B_w = np.asarray(B_w, dtype=f32)
    A_w = np.asarray(A_w, dtype=f32)
    A_b = np.asarray(A_b, dtype=f32)

    # Weights, pre-scaled for the shared-psum 2^15 convention:
    #   main:  (Bh*2^8 fp16) @ (xh*2^7 fp16)
    #   corr:  (Bh*2^6 fp8) @ (xl*2^9 fp8)  +  (Bl*2^17 fp8) @ (x*2^-2 fp8)
    BT = np.ascontiguousarray(B_w.T).astype(f32) * np.float32(2.0**8)
    Bh16 = BT.astype(f16)                       # [IN_F, RANK] at 2^8
    Bl_s = BT - Bh16.astype(f32)
    w0 = (Bh16.astype(f32) * np.float32(0.25)).astype(e4)
    w1 = (Bl_s * np.float32(512.0)).astype(e4)
    bh = np.ascontiguousarray(
        Bh16.reshape(KT1, P, MR, P).transpose(1, 0, 2, 3))
    w0p = w0.reshape(KT1, P, MR, P).transpose(1, 0, 2, 3)
    w1p = w1.reshape(KT1, P, MR, P).transpose(1, 0, 2, 3)
    w8 = np.ascontiguousarray(np.stack([w0p, w1p], axis=3))  # [P,KT1,MR,2,P]

    AwT = np.ascontiguousarray(A_w.T)                        # [RANK, OUT_F]
    Aw = np.ascontiguousarray(
        AwT.astype(bf16).reshape(MR, P, NT2, 512).transpose(1, 2, 0, 3))

    bias_rep = np.ascontiguousarray(
        np.broadcast_to(A_b.astype(f32, copy=False), (P, OUT_F)))

    in_maps = []
    for c in range(NCORES):
        xT = np.ascontiguousarray(input[c * TPC : (c + 1) * TPC].T).astype(f32)
        xTs = xT * np.float32(2.0**7)
        xh16 = xTs.astype(f16)
        xl_s = xTs - xh16.astype(f32)
        m0 = (xl_s * np.float32(4.0)).astype(e4)
        m1 = (xTs * np.float32(2.0**-9)).astype(e4)
        xh = np.ascontiguousarray(
            xh16.reshape(KT1, P, NT1, 512).transpose(1, 2, 0, 3))
        m0p = m0.reshape(KT1, P, NT1, 512).transpose(1, 2, 0, 3)
        m1p = m1.reshape(KT1, P, NT1, 512).transpose(1, 2, 0, 3)
        m8 = np.ascontiguousarray(np.stack([m0p, m1p], axis=3))
        in_maps.append(
            {"xh": xh, "m8": m8, "bh": bh, "w8": w8, "aw": Aw, "bias": bias_rep}
        )

    res = bass_utils.run_bass_kernel_spmd(nc, in_maps, core_ids=list(range(NCORES)))
    out = np.concatenate([res.results[c]["out"] for c in range(NCORES)], axis=0)
    return out.astype(np.float32, copy=False)
